# revision 35
# baseline (speedup 1.0000x reference)
"""Trainium2 Bass kernel for nn_MultiHeadAttention_22883585753377 (v3).

Reference semantics (torch legacy): softmax over the HEADS axis (dim=1) of
the [B,H,S,S] score tensor, scale = sqrt(KEY_DIM)=32.

Sharding: 8 cores = (batch b, query-quarter r). Each core handles b = c//4,
512 query rows, all 16 heads.

v3 structure:
  - V projection is sharded 4-way across each batch group (each core
    projects only its own 512 keys) and exchanged with ONE AllGather
    (replica groups [[0-3],[4-7]]). Key order is permutation-invariant, so
    cores consume the gathered chunks in rank order; the own chunk is read
    back too (simplest uniform program). The gather latency hides behind
    the Q/K projections and the softmax pipeline running ahead of AV.
  - K projection stays full and local (scores need K immediately; a gather
    here exposes the collective latency on the critical path).
  - Softmax chain: exp on ACT (PSUM src, 4-head groups), denominator tree +
    normalize-multiply on DVE, reciprocal via Ln/-Exp on ACT one iteration
    late. AV into 4 PSUM banks col-tiled (M=64 pairs run concurrently).
  - Projection bias adds ride the PE (ones-row outer products), so ACT/DVE
    only do plain PSUM->SBUF copies.
"""

import numpy as np

B = 2
S = 1024 * 2
D = 1024
H = 16
DH = 64
SQ = 512  # query rows per core
QH = 256  # q processed per half
KC = 128  # k-chunk (partition dim of scores^T tiles)
NKC = S // KC  # 16
KC4 = 512  # projection / shard chunk
NKC4 = S // KC4  # 4
SCALE = 1.0 / 32.0  # 1/sqrt(KEY_DIM)
LAG1 = 2
LAG2 = 3
WBUF1 = 4  # w-ring depth stage 1 (run-ahead while V arrives)
WBUF2 = 4
SCORE_MODE = 1  # 0: one K=128 zero-padded MM per pair; 1: two row-tiled K=64 MMs

# score_mode=1 packs each 4-head group's scores as [hh0-pair0, hh0-pair1 |
# hh1-pair0, hh1-pair1] so the two concurrent row-groups write DIFFERENT
# PSUM banks (same-bank cross-row-group writes crash the device).  BLK[h]
# is the e/w column block holding head h.
BLK = [0] * H
for _g in range(4):
    BLK[4 * _g + 0] = 4 * _g + 0
    BLK[4 * _g + 2] = 4 * _g + 1
    BLK[4 * _g + 1] = 4 * _g + 2
    BLK[4 * _g + 3] = 4 * _g + 3

_CACHE = {}


def _legalize_waits(nc):
    """This container's walrus encodes at most ONE semaphore wait per
    instruction; Tile emits up to ~10. Split the excess onto same-engine nops
    inserted immediately before the instruction."""
    import bass_rust

    ctr = [0]
    for bb in nc.main_func.blocks:
        insts = list(bb.instructions)
        out = []
        changed = False
        for ins in insts:
            si = ins.sync_info
            waits = list(si.on_wait) if si is not None and si.on_wait else []
            if len(waits) > 1:
                changed = True
                upd = list(si.on_update) if si.on_update else []
                for w in waits[:-1]:
                    ctr[0] += 1
                    nop = bass_rust.InstNoOp(
                        name=f"I-wsplit-{ctr[0]}", ins=[], outs=[]
                    )
                    nop.engine = ins.engine
                    nop.bass_nofuse = True
                    nop.sync_info = bass_rust.SyncInfo(on_wait=[w], on_update=[])
                    out.append(nop)
                ins.sync_info = bass_rust.SyncInfo(
                    on_wait=[waits[-1]], on_update=upd
                )
            out.append(ins)
        if changed:
            bb.instructions = out


def _bcast_cols(ap, rep, seg):
    import dataclasses

    a = [tuple(x) for x in ap.ap]
    assert a[-1][1] == seg, a
    return dataclasses.replace(ap, ap=[a[0], (0, rep), a[-1]])


def _split_cols(ap, rep, seg):
    import dataclasses

    a = [tuple(x) for x in ap.ap]
    assert a[-1] == (1, rep * seg), a
    return dataclasses.replace(ap, ap=[a[0], (seg, rep), (1, seg)])


def _ins_dim(ap, stride, rep):
    import dataclasses

    a = [tuple(x) for x in ap.ap]
    assert len(a) == 2, a
    return dataclasses.replace(ap, ap=[a[0], (stride, rep), a[-1]])


def _src3d(ap, nchunk, rows, cols, row_stride):
    """View a DRAM [nchunk*rows, cols] AP as [rows, nchunk, cols] so one DMA
    fills an SBUF [128, nchunk, cols] tile (partition-major dst order)."""
    import dataclasses

    a = [tuple(x) for x in ap.ap]
    assert len(a) == 2 and a[1][0] == 1, a
    return dataclasses.replace(
        ap, ap=[(row_stride, rows), (rows * row_stride, nchunk), (1, cols)]
    )


def _build(legalize=True, score_mode=SCORE_MODE):
    import concourse.bass as bass
    import concourse.mybir as mybir
    import concourse.tile as tile

    bf16 = mybir.dt.bfloat16
    f32 = mybir.dt.float32
    AF = mybir.ActivationFunctionType

    nc = bass.Bass()

    # --- I/O ---------------------------------------------------------------
    qT_d = nc.dram_tensor("qT", [D, SQ], bf16, kind="ExternalInput")
    kT_d = nc.dram_tensor("kT", [D, S], bf16, kind="ExternalInput")
    vTs_d = nc.dram_tensor("vTs", [D, KC4], bf16, kind="ExternalInput")
    wq_d = nc.dram_tensor("wq", [D, D], bf16, kind="ExternalInput")
    wk_d = nc.dram_tensor("wk", [D, D], bf16, kind="ExternalInput")
    wv_d = nc.dram_tensor("wv", [D, D], bf16, kind="ExternalInput")
    wo_d = nc.dram_tensor("wo", [D, D], bf16, kind="ExternalInput")
    bq_d = nc.dram_tensor("bq", [1, D], bf16, kind="ExternalInput")
    bk_d = nc.dram_tensor("bk", [1, D], bf16, kind="ExternalInput")
    bv_d = nc.dram_tensor("bv", [1, D], bf16, kind="ExternalInput")
    bo_d = nc.dram_tensor("bo", [1, D], bf16, kind="ExternalInput")
    out_d = nc.dram_tensor("out", [SQ, D], f32, kind="ExternalOutput")

    RG = [[0, 1, 2, 3], [4, 5, 6, 7]]

    with tile.TileContext(nc) as tc:
        with (
            tc.tile_pool(name="persist", bufs=1) as persist,
            tc.tile_pool(name="consts", bufs=1) as consts,
            tc.tile_pool(name="ccdram", bufs=1, space="DRAM") as ccdram,
        ):
            KT = [
                [persist.tile([128, KC4], bf16, tag=f"KT{c}_{p}", name=f"KT{c}_{p}")
                 for p in range(8)]
                for c in range(NKC4)
            ]
            V = [persist.tile([128, D], bf16, tag=f"V{s}", name=f"V{s}") for s in range(16)]
            QTb = persist.tile([128, H * SQ], bf16, tag="QTb", name="QTb")
            OT = [
                [persist.tile([128, QH], bf16, tag=f"OT{qh}_{p}", name=f"OT{qh}_{p}")
                 for p in range(8)]
                for qh in range(2)
            ]

            ones = consts.tile([1, KC4], bf16)
            nc.vector.memset(ones[:], 1.0)
            bq_s = consts.tile([1, D], bf16, tag="bq")
            bk_s = consts.tile([1, D], bf16, tag="bk")
            bv_s = consts.tile([1, D], bf16, tag="bv")
            bo_s = consts.tile([1, D], bf16, tag="bo")
            nc.sync.dma_start(bq_s[:], bq_d[:])
            nc.sync.dma_start(bk_s[:], bk_d[:])
            nc.sync.dma_start(bv_s[:], bv_d[:])
            nc.sync.dma_start(bo_s[:], bo_d[:])

            ccinv = ccdram.tile([128, 4 * D], bf16, name="ccinv")
            ccoutv = ccdram.tile([512, 4 * D], bf16, name="ccoutv")

            # ---------------- Phase A: V (sharded) + Q projections ---------
            # wk/kraw live OUTSIDE the early pool: their SBUF must not alias
            # the early tiles, else the wk DMAs would wait for Q-proj to
            # finish and then crawl through the collective's DMA window.
            from contextlib import ExitStack

            kstack = ExitStack()
            wkp = kstack.enter_context(tc.tile_pool(name="wk_sb", bufs=1))
            krawp = kstack.enter_context(tc.tile_pool(name="k_raw", bufs=3))
            wkall = wkp.tile([128, 8 * D], bf16, tag="wkall", name="wkall")
            kr = [None] * NKC4

            def dma_k_chunk(c):
                kt = krawp.tile([128, 8 * KC4], bf16, tag="kraw", name="kraw")
                for d in range(8):
                    nc.sync.dma_start(
                        kt[:, d * KC4 : (d + 1) * KC4],
                        kT_d[d * 128 : (d + 1) * 128, c * KC4 : (c + 1) * KC4],
                    )
                return kt

            with tc.tile_pool(name="wearly", bufs=1) as wearly:
                wvall = wearly.tile([128, 8 * D], bf16, tag="wvall", name="wvall")
                wqall = wearly.tile([128, 8 * D], bf16, tag="wqall", name="wqall")
                vraw = wearly.tile([128, 8 * KC4], bf16, tag="vraw", name="vraw")
                qraw = wearly.tile([128, 8 * SQ], bf16, tag="qraw", name="qraw")
                vstg = wearly.tile([128, 4 * D], bf16, tag="vstg", name="vstg")

                # V path first on the sync queue; Q path trickles on ACT queue.
                # Per-128-row dma_starts spread across the DMA engines (a
                # single big dma_start runs on ONE engine and serializes).
                for d in range(8):
                    nc.sync.dma_start(vraw[:, d * KC4 : (d + 1) * KC4], vTs_d[d * 128 : (d + 1) * 128, :])
                for d in range(8):
                    nc.sync.dma_start(wvall[:, d * D : (d + 1) * D], wv_d[d * 128 : (d + 1) * 128, :])
                for d in range(8):
                    nc.scalar.dma_start(qraw[:, d * SQ : (d + 1) * SQ], qT_d[d * 128 : (d + 1) * 128, :])
                    nc.scalar.dma_start(wqall[:, d * D : (d + 1) * D], wq_d[d * 128 : (d + 1) * 128, :])
                # K-path inputs right behind the V path, ahead of the
                # collective's DMA-heavy window.
                for d in range(8):
                    nc.sync.dma_start(wkall[:, d * D : (d + 1) * D], wk_d[d * 128 : (d + 1) * 128, :])
                kr[0] = dma_k_chunk(0)
                kr[1] = dma_k_chunk(1)
                kr[2] = dma_k_chunk(2)

                with tc.tile_pool(name="projA_ps", bufs=2, space="PSUM") as projp:
                    # V projection of the local 512-key shard -> vstg.
                    for sv in range(4):
                        for f2 in range(2):
                            pv = projp.tile([128, KC4], f32, tag="pj")
                            for d in range(8):
                                nc.tensor.matmul(
                                    pv[:],
                                    vraw[:, d * KC4 + sv * 128 : d * KC4 + (sv + 1) * 128],
                                    wvall[:, d * D + f2 * 512 : d * D + (f2 + 1) * 512],
                                    start=(d == 0),
                                    stop=False,
                                )
                            nc.tensor.matmul(
                                pv[:],
                                ones[0:1, 0:128],
                                bv_s[0:1, f2 * 512 : (f2 + 1) * 512],
                                start=False,
                                stop=True,
                            )
                            nc.vector.tensor_copy(
                                vstg[:, sv * D + f2 * 512 : sv * D + (f2 + 1) * 512],
                                pv[:],
                            )
                    # ccinv write + V readback ride the (otherwise idle)
                    # GpSimd DMA queue so they never block the K-path input
                    # DMAs on the sync queue behind the collective.
                    nc.gpsimd.dma_start(ccinv[:], vstg[:])
                    nc.gpsimd.collective_compute(
                        "AllGather",
                        mybir.AluOpType.bypass,
                        replica_groups=RG,
                        ins=[ccinv.opt()],
                        outs=[ccoutv.opt()],
                    )

                    # Q projection -> QTb.
                    for f in range(8):
                        ps = projp.tile([128, SQ], f32, tag="pj")
                        for d in range(8):
                            nc.tensor.matmul(
                                ps[:],
                                wqall[:, d * D + f * 128 : d * D + (f + 1) * 128],
                                qraw[:, d * SQ : (d + 1) * SQ],
                                start=(d == 0),
                                stop=False,
                            )
                        nc.tensor.matmul(
                            ps[:],
                            bq_s[0:1, f * 128 : (f + 1) * 128],
                            ones[0:1, :],
                            start=False,
                            stop=True,
                        )
                        nc.scalar.copy(
                            QTb[0:64, (2 * f) * SQ : (2 * f + 1) * SQ], ps[0:64, :]
                        )
                        nc.scalar.copy(
                            QTb[64:128, (2 * f + 1) * SQ : (2 * f + 2) * SQ],
                            ps[64:128, :],
                        )
                    if score_mode == 0:
                        for h in range(16):
                            r = (h % 2) * 64
                            nc.vector.memset(
                                QTb[64 - r : 128 - r, h * SQ : (h + 1) * SQ], 0.0
                            )

                # V read back (own chunk included; key order is irrelevant).
                for g in range(4):
                    for j in range(4):
                        nc.gpsimd.dma_start(
                            V[4 * g + j][:],
                            ccoutv[g * 128 : (g + 1) * 128, j * D : (j + 1) * D],
                        )

            # ---------------- fused attention helpers ----------------
            def attn_scores(qh, kc, escp, eexpp, interleave=None):
                """Scores + exp for one (qh, kc): 4 groups of 4 heads.
                `interleave(g)` emits extra PE work between score groups."""
                c, rr = kc // 4, kc % 4
                e = eexpp.tile([128, H * QH], bf16, tag="e")
                for g in range(4):
                    sc = escp.tile([128, 4 * QH], f32, tag="sc")
                    if score_mode == 1:
                        # 4 row-tiled K=64 MMs: emission order interleaves
                        # the two row groups (concurrent sub-arrays); each
                        # row group targets its own PSUM bank.  Bank A
                        # (cols 0:512) = hh0 heads {4g, 4g+2}, bank B
                        # (cols 512:1024) = hh1 heads {4g+1, 4g+3}.
                        for pp in range(2):
                            p = 2 * g + pp
                            for hh in range(2):
                                h = 4 * g + 2 * pp + hh
                                blk = 2 * hh + pp  # local block in sc
                                nc.tensor.matmul(
                                    sc[:, blk * QH : (blk + 1) * QH],
                                    KT[c][p][hh * 64 : (hh + 1) * 64, rr * 128 : (rr + 1) * 128],
                                    QTb[hh * 64 : (hh + 1) * 64,
                                        h * SQ + qh * QH : h * SQ + qh * QH + QH],
                                    start=(pp == 0),
                                    stop=(pp == 1),
                                    skip_group_check=True,
                                )
                    if score_mode == 0:
                        for pp in range(2):
                            p = 2 * g + pp
                            mov = _ins_dim(
                                QTb[:, 2 * p * SQ + qh * QH : 2 * p * SQ + qh * QH + QH],
                                SQ, 2,
                            )
                            nc.tensor.matmul(
                                sc[:, pp * 2 * QH : (pp + 1) * 2 * QH],
                                KT[c][p][:, rr * 128 : (rr + 1) * 128],
                                mov,
                                start=True,
                                stop=True,
                            )
                    nc.scalar.activation(
                        e[:, g * 4 * QH : (g + 1) * 4 * QH],
                        sc[:],
                        AF.Exp,
                        scale=SCALE,
                    )
                    if interleave is not None:
                        interleave(g)
                return e

            def attn_tree(e, emid):
                t1 = emid.tile([128, 8 * QH], bf16, tag="t1", bufs=1)
                nc.vector.tensor_add(t1[:], e[:, : 8 * QH], e[:, 8 * QH :])
                t2 = emid.tile([128, 4 * QH], bf16, tag="t2", bufs=1)
                nc.vector.tensor_add(t2[:], t1[:, : 4 * QH], t1[:, 4 * QH :])
                t3 = emid.tile([128, 2 * QH], bf16, tag="t3", bufs=1)
                nc.vector.tensor_add(t3[:], t2[:, : 2 * QH], t2[:, 2 * QH :])
                den = emid.tile([128, QH], f32, tag="den")
                nc.vector.tensor_add(den[:], t3[:, :QH], t3[:, QH:])
                return den

            def attn_finish(e, den, emid, ewtsp):
                lden = emid.tile([128, QH], f32, tag="lden", bufs=1)
                nc.scalar.activation(lden[:], den[:], AF.Ln)
                r16 = emid.tile([128, QH], bf16, tag="r16")
                nc.scalar.activation(r16[:], lden[:], AF.Exp, scale=-1.0)
                w = ewtsp.tile([128, H * QH], bf16, tag="w")
                rb = _bcast_cols(r16[:], H, QH)
                nc.vector.tensor_mul(
                    _split_cols(w[:, :], H, QH),
                    _split_cols(e[:, :], H, QH),
                    rb,
                )
                return w

            def attn_av(oacc, kc, w):
                blk = BLK if score_mode == 1 else list(range(H))
                for j in range(8):
                    cs = slice((j // 4) * QH, (j // 4 + 1) * QH)
                    for hh in range(2):
                        h = 2 * j + hh
                        nc.tensor.matmul(
                            oacc[j % 4][hh * 64 : (hh + 1) * 64, cs],
                            V[kc][:, h * 64 : (h + 1) * 64],
                            w[:, blk[h] * QH : (blk[h] + 1) * QH],
                            start=(kc == 0 and j < 4),
                            stop=(kc == NKC - 1),
                            skip_group_check=True,
                        )

            def oacc_flush(qh, oacc):
                for j in range(8):
                    cs = slice((j // 4) * QH, (j // 4 + 1) * QH)
                    nc.scalar.copy(OT[qh][j][:], oacc[j % 4][:, cs])

            # ---------------- Stage 1: K proj (streamed) + attention qh=0 --
            if True:
                with (
                    tc.tile_pool(name="sc_ps", bufs=2, space="PSUM") as scp,
                    tc.tile_pool(name="oacc_ps", bufs=1, space="PSUM") as oaccp,
                    tc.tile_pool(name="exp_sb", bufs=2) as expp,
                    tc.tile_pool(name="wts_sb", bufs=1 + WBUF1) as wtsp,
                    tc.tile_pool(name="mid_sb", bufs=2) as mid,
                ):
                    def proj_k_ff(c, kra, ff):
                        """K projection for 512-wide chunk c, one 128-feature
                        block ff (9 MMs + a copy split across ACT/DVE). Its
                        PSUM rides the scores pool's double-buffered slots
                        (PSUM budget: scores 4 banks + oacc 4)."""
                        psfull = scp.tile([128, 4 * QH], f32, tag="sc", name="sc")
                        ps = psfull[:, :KC4]
                        for d in range(8):
                            nc.tensor.matmul(
                                ps[:],
                                wkall[:, d * D + ff * 128 : d * D + (ff + 1) * 128],
                                kra[:, d * KC4 : (d + 1) * KC4],
                                start=(d == 0),
                                stop=False,
                            )
                        nc.tensor.matmul(
                            ps[:],
                            bk_s[0:1, ff * 128 : (ff + 1) * 128],
                            ones[0:1, :],
                            start=False,
                            stop=True,
                        )
                        if ff % 2 == 0:
                            nc.scalar.copy(KT[c][ff][:], ps[:])
                        else:
                            nc.vector.tensor_copy(KT[c][ff][:], ps[:])

                    # Only chunk 0 upfront; chunks 1-3 interleave at two
                    # ff-blocks per kc: chunk c done by kc=4(c-1)+4 = 4c,
                    # exactly when the scores first need it.
                    for ff in range(8):
                        proj_k_ff(0, kr[0], ff)
                    il_sched = {}  # kc -> (chunk, ff_base)
                    for i in range(12):
                        il_sched[i] = (1 + i // 4, 2 * (i % 4))

                    oaccA = [
                        oaccp.tile([128, 2 * QH], f32, tag=f"oA{i}", name=f"oA{i}")
                        for i in range(4)
                    ]
                    ering = [None] * NKC
                    dring = [None] * NKC
                    wring = [None] * NKC
                    for kc in range(NKC):
                        # Interleave two ff-blocks of remaining K projection
                        # between this iteration's score groups.
                        if kc == 0:
                            kr[3] = dma_k_chunk(3)
                        il = None
                        if kc in il_sched:
                            c_il, ff_il = il_sched[kc]

                            def il(g, c=c_il, ffb=ff_il):
                                if g == 1:
                                    proj_k_ff(c, kr[c], ffb)
                                elif g == 3:
                                    proj_k_ff(c, kr[c], ffb + 1)

                        e = attn_scores(0, kc, scp, expp, interleave=il)
                        ering[kc] = e
                        dring[kc] = attn_tree(e, mid)
                        if kc >= 1:
                            wring[kc - 1] = attn_finish(
                                ering[kc - 1], dring[kc - 1], mid, wtsp
                            )
                        if kc >= 1 + LAG1:
                            attn_av(oaccA, kc - 1 - LAG1, wring[kc - 1 - LAG1])
                    wring[NKC - 1] = attn_finish(
                        ering[NKC - 1], dring[NKC - 1], mid, wtsp
                    )
                    for kc in range(NKC - 1 - LAG1, NKC):
                        attn_av(oaccA, kc, wring[kc])
                    oacc_flush(0, oaccA)
                kstack.close()  # free wk/kraw SBUF for stage 2 pools

            # ---------------- Stage 2: attention qh=1 ----------------
            with tc.tile_pool(name="wot_sb", bufs=1) as wot:
                woall = wot.tile([128, 8 * D], bf16, tag="woall", name="woall")
                for j in range(8):
                    nc.sync.dma_start(woall[:, j * D : (j + 1) * D], wo_d[j * 128 : (j + 1) * 128, :])

                with (
                    tc.tile_pool(name="sc2_ps", bufs=2, space="PSUM") as scp2,
                    tc.tile_pool(name="oacc2_ps", bufs=1, space="PSUM") as oaccp2,
                    tc.tile_pool(name="exp2_sb", bufs=4) as expp2,
                    tc.tile_pool(name="wts2_sb", bufs=1 + WBUF2) as wtsp2,
                    tc.tile_pool(name="mid2_sb", bufs=3) as mid2,
                ):
                    oaccB = [
                        oaccp2.tile([128, 2 * QH], f32, tag=f"oB{i}", name=f"oB{i}")
                        for i in range(4)
                    ]
                    ering2 = [None] * NKC
                    dring2 = [None] * NKC
                    wring2 = [None] * NKC
                    for kc in range(NKC):
                        e = attn_scores(1, kc, scp2, expp2)
                        ering2[kc] = e
                        dring2[kc] = attn_tree(e, mid2)
                        if kc >= 1:
                            wring2[kc - 1] = attn_finish(
                                ering2[kc - 1], dring2[kc - 1], mid2, wtsp2
                            )
                        if kc >= 1 + LAG2:
                            attn_av(oaccB, kc - 1 - LAG2, wring2[kc - 1 - LAG2])
                    wring2[NKC - 1] = attn_finish(
                        ering2[NKC - 1], dring2[NKC - 1], mid2, wtsp2
                    )
                    for kc in range(NKC - 1 - LAG2, NKC):
                        attn_av(oaccB, kc, wring2[kc])
                    oacc_flush(1, oaccB)

                # Stage 3: output projection.
                with (
                    tc.tile_pool(name="pO1", bufs=2, space="PSUM") as pO1,
                    tc.tile_pool(name="osb", bufs=2) as osb,
                ):
                    for q4 in range(4):
                        qh, qr = q4 // 2, q4 % 2
                        po = pO1.tile([128, D], f32, tag="po")
                        for j in range(8):
                            for f2 in range(2):
                                nc.tensor.matmul(
                                    po[:, f2 * 512 : (f2 + 1) * 512],
                                    OT[qh][j][:, qr * 128 : (qr + 1) * 128],
                                    woall[:, j * D + f2 * 512 : j * D + (f2 + 1) * 512],
                                    start=(j == 0),
                                    stop=False,
                                )
                        for f2 in range(2):
                            nc.tensor.matmul(
                                po[:, f2 * 512 : (f2 + 1) * 512],
                                ones[0:1, 0:128],
                                bo_s[0:1, f2 * 512 : (f2 + 1) * 512],
                                start=False,
                                stop=True,
                            )
                        ob = osb.tile([128, D], f32, tag="ob")
                        nc.vector.tensor_copy(ob[:], po[:])
                        nc.sync.dma_start(out_d[q4 * 128 : (q4 + 1) * 128, :], ob[:])

    if legalize:
        _legalize_waits(nc)
    return nc


def _prep_inputs(inputs):
    import ml_dtypes

    bf16 = ml_dtypes.bfloat16
    q = np.asarray(inputs["queries"], np.float32)
    k = np.asarray(inputs["keys"], np.float32)
    v = np.asarray(inputs["values"], np.float32)
    Wq = np.asarray(inputs["Wq"], np.float32).astype(bf16)
    Wk = np.asarray(inputs["Wk"], np.float32).astype(bf16)
    Wv = np.asarray(inputs["Wv"], np.float32).astype(bf16)
    Wo = np.asarray(inputs["Wo"], np.float32).astype(bf16)
    bq = np.asarray(inputs["bq"], np.float32).astype(bf16).reshape(1, D)
    bk = np.asarray(inputs["bk"], np.float32).astype(bf16).reshape(1, D)
    bv = np.asarray(inputs["bv"], np.float32).astype(bf16).reshape(1, D)
    bo = np.asarray(inputs["bo"], np.float32).astype(bf16).reshape(1, D)

    kT = [np.ascontiguousarray(k[b].T).astype(bf16) for b in range(B)]

    in_maps = []
    for c in range(8):
        b, r = c // 4, c % 4
        qT = np.ascontiguousarray(q[b, r * SQ : (r + 1) * SQ, :].T).astype(bf16)
        vTs = np.ascontiguousarray(v[b, r * KC4 : (r + 1) * KC4, :].T).astype(bf16)
        in_maps.append(
            {
                "qT": qT,
                "kT": kT[b],
                "vTs": vTs,
                "wq": Wq,
                "wk": Wk,
                "wv": Wv,
                "wo": Wo,
                "bq": bq,
                "bk": bk,
                "bv": bv,
                "bo": bo,
            }
        )
    return in_maps


def run(inputs, trace=False, trace_kwargs=None):
    """Build (cached), run on 8 cores, return (output, BassKernelResults)."""
    from concourse.bass_utils import run_bass_kernel_spmd

    if "nc" not in _CACHE:
        _CACHE["nc"] = _build()
    nc = _CACHE["nc"]
    in_maps = _prep_inputs(inputs)
    res = run_bass_kernel_spmd(
        nc,
        in_maps,
        core_ids=list(range(8)),
        trace=trace,
        **(trace_kwargs or {}),
    )
    out = np.empty((B, S, D), np.float32)
    for c in range(8):
        b, r = c // 4, c % 4
        out[b, r * SQ : (r + 1) * SQ, :] = res.results[c]["out"]
    return out, res


def kernel(**inputs) -> np.ndarray:
    out, _ = run(inputs, trace=False)
    return out
